# revision 1
# baseline (speedup 1.0000x reference)
"""Trainium2 Bass kernel for GQA multi-head attention (B=2,S=2048,HID=2048,H=32,KVH=8,D=64).

Sharding: 8 cores = 2 (batch) x 4 (kv-head groups). Each core handles one batch
element and 2 kv heads (= 8 q heads), computes its partial o_proj output
(contracting only its 512 attention features), host sums 4 partials per batch.

Device layouts (host pre-marshalled):
  hsT   [HID, S]   transposed hidden states for this core's batch
  cosT2 [128, S]   cos.T replicated for the 2 heads packed per partition-block
  sinT2 [128, S]   sin.T replicated
  rotm  [128, 128] transposed rotate_half matrix (R.T with R q = rotate_half(q))
  wqT   [HID, 512] Wq rows for this core's q heads (perm order), transposed
  wkT   [HID, 128] Wk rows for 2 kv heads, transposed
  wvT   [HID, 128]
  woT   [512, HID] Wo columns for this core's features (perm order), transposed
  out   [S, HID]   partial output (sum over cores with same batch on host)

Head perm order per core: local q heads [0,4,1,5,2,6,3,7] so each 128-partition
q tile j holds (head needing kv0 at partitions 0:64, head needing kv1 at 64:128),
aligning with kT/vT partition halves for row-packed matmuls.

Softmax: scores are O(10) so exp without max-subtraction is safe in fp32; the
row sum rides along as a ones-column in the attn@V weights; normalization is a
reciprocal + PE outer-product broadcast + elementwise multiply.
"""

import sys

if "/opt/trn_rl_repo" not in sys.path:
    sys.path.insert(0, "/opt/trn_rl_repo")

import numpy as np

B, S, HID = 2, 2048, 2048
H, KVH, D = 32, 8, 64
NCORES = 8

PERM_LOCAL = [0, 4, 1, 5, 2, 6, 3, 7]

_NC_CACHE = {}


def _build_nc():
    import concourse.bass as bass
    import concourse.mybir as mybir
    from concourse import bacc
    from concourse.tile import TileContext
    from concourse.masks import make_identity
    from contextlib import ExitStack

    f32 = mybir.dt.float32
    f32r = mybir.dt.float32r
    Exp = mybir.ActivationFunctionType.Exp
    mult = mybir.AluOpType.mult
    add = mybir.AluOpType.add

    nc = bacc.Bacc(None, target_bir_lowering=False)

    hsT = nc.declare_dram_parameter("hsT", [HID, S], f32r, isOutput=False)
    cosT2 = nc.declare_dram_parameter("cosT2", [128, S], f32, isOutput=False)
    sinT2 = nc.declare_dram_parameter("sinT2", [128, S], f32, isOutput=False)
    rotm = nc.declare_dram_parameter("rotm", [128, 128], f32r, isOutput=False)
    wqT = nc.declare_dram_parameter("wqT", [HID, 512], f32r, isOutput=False)
    wkT = nc.declare_dram_parameter("wkT", [HID, 128], f32r, isOutput=False)
    wvT = nc.declare_dram_parameter("wvT", [HID, 128], f32r, isOutput=False)
    woT = nc.declare_dram_parameter("woT", [512, HID], f32r, isOutput=False)
    out = nc.declare_dram_parameter("out", [S, HID], f32, isOutput=True)

    KT = HID // 128  # 16 contraction k-tiles for projections
    SCA = 256        # phase-A s-chunk width
    NSCA = S // SCA  # 8
    TT = S // 128    # 16 t-tiles
    SCB = 512        # phase-B s-chunk width
    NSCB = S // SCB  # 4

    with TileContext(nc) as tc, ExitStack() as ctx:
        # ---------------- pools ----------------
        persist = ctx.enter_context(tc.tile_pool(name="persist", bufs=1))
        qT_sb = persist.tile([128, 4, S], f32r)       # q' transposed, 4 e-tiles
        kT_sb = persist.tile([128, S], f32r)          # k' transposed (2 kv heads)
        v_sb = persist.tile([128, TT, 130], f32r)     # v[t,d] per t-tile + ones cols
        oT_sb = persist.tile([128, 4, S], f32r)       # normalized attn out ^T
        ones_sb = persist.tile([128, 256], f32r)
        rot_sb = persist.tile([128, 128], f32r)

        # PSUM: scores pool 3x2banks + small pool 2x1bank = 8 banks
        sp = ctx.enter_context(tc.tile_pool(name="sp", bufs=3, space="PSUM"))
        op = ctx.enter_context(tc.tile_pool(name="op", bufs=2, space="PSUM"))

        # phase-B P^T staging
        ptp = ctx.enter_context(tc.tile_pool(name="ptp", bufs=2))
        rrp = ctx.enter_context(tc.tile_pool(name="rrp", bufs=2))
        obst = ctx.enter_context(tc.tile_pool(name="obst", bufs=2))

        onesf = persist.tile([128, 256], f32)
        nc.vector.memset(onesf, 1.0)
        # masks for the two normalize outer-products: row0 cols0:128 -> head A
        # mask (1 on partitions 0:64), cols 128:256 -> head B mask (1 on 64:128)
        nc.vector.memset(onesf[0:1, 64:192], 0.0)
        nc.vector.tensor_copy(ones_sb, onesf[:, 0:256])
        nc.sync.dma_start(out=rot_sb, in_=rotm[:, :])
        # ones columns of v tiles (col 64 for head A, col 129 for head B)
        nc.vector.tensor_copy(v_sb[:, :, 64], onesf[:, 0:16])
        nc.vector.tensor_copy(v_sb[:, :, 129], onesf[:, 0:16])

        # ---------------- phase A: projections + RoPE + v transpose -------------
        actx = ExitStack()
        pcs = actx.enter_context(tc.tile_pool(name="pcs", bufs=1))
        cos_sb = pcs.tile([128, S], f32)
        sin_sb = pcs.tile([128, S], f32)
        nc.sync.dma_start(out=cos_sb, in_=cosT2[:, :])
        nc.sync.dma_start(out=sin_sb, in_=sinT2[:, :])

        hsp = actx.enter_context(tc.tile_pool(name="hsp", bufs=2))
        ropep = actx.enter_context(tc.tile_pool(name="ropep", bufs=2))

        a1ctx = ExitStack()
        pkv = a1ctx.enter_context(tc.tile_pool(name="pkv", bufs=1))
        wk_sb = pkv.tile([128, KT, 128], f32r)
        wv_sb = pkv.tile([128, KT, 128], f32r)
        ident = pkv.tile([128, 128], f32)
        nc.sync.dma_start(out=wk_sb, in_=wkT.rearrange("(t p) e -> p t e", p=128))
        nc.sync.dma_start(out=wv_sb, in_=wvT.rearrange("(t p) e -> p t e", p=128))
        make_identity(nc, ident)
        vstg = a1ctx.enter_context(tc.tile_pool(name="vstg", bufs=2))

        def rope_drain(ps, rot_ps, dst, sl):
            """ps: [128, SCA] psum q/k tile; rot_ps: [128, SCA] spare psum slot.
            Computes dst[:, sl] = ps*cos + (R@ps)*sin."""
            qsb = ropep.tile([128, SCA], f32r, name="qsb", tag="qsb")
            nc.scalar.copy(qsb, ps)
            nc.tensor.matmul(rot_ps, rot_sb, qsb, start=True, stop=True)
            nc.vector.tensor_tensor(out=dst[:, sl], in0=qsb, in1=cos_sb[:, sl],
                                    op=mult)
            shs = ropep.tile([128, SCA], f32, name="shs", tag="shs")
            nc.vector.tensor_tensor(out=shs, in0=rot_ps, in1=sin_sb[:, sl], op=mult)
            nc.vector.tensor_tensor(out=dst[:, sl], in0=dst[:, sl], in1=shs, op=add)

        # K/V first (attention needs all of k/v), then Q
        for sc in range(NSCA):
            sl = slice(sc * SCA, (sc + 1) * SCA)
            hs_sb = hsp.tile([128, KT, SCA], f32r, name="hs_sb")
            nc.sync.dma_start(
                out=hs_sb, in_=hsT[:, sl].rearrange("(t p) s -> p t s", p=128))
            kv_ps = sp.tile([128, 2, 512], f32, name="kv_ps", tag="sp")
            for ki in range(KT):
                nc.tensor.matmul(kv_ps[:, 0, 0:SCA], wk_sb[:, ki, :],
                                 hs_sb[:, ki, :], start=ki == 0, stop=ki == KT - 1)
            for ki in range(KT):
                nc.tensor.matmul(kv_ps[:, 0, SCA : 2 * SCA], wv_sb[:, ki, :],
                                 hs_sb[:, ki, :], start=ki == 0, stop=ki == KT - 1)
            # k: RoPE into kT_sb (rot output borrows slot 1 of same psum tile)
            rope_drain(kv_ps[:, 0, 0:SCA], kv_ps[:, 1, 0:SCA], kT_sb, sl)
            # v: stage, transpose 128-blocks into v_sb[t, d] layout
            vt_sb = vstg.tile([128, SCA], f32, name="vt_sb")
            nc.scalar.copy(vt_sb, kv_ps[:, 0, SCA : 2 * SCA])
            for i in range(SCA // 128):
                tt = (sc * SCA) // 128 + i
                tps = op.tile([128, 512], f32, name="tps", tag="o")
                nc.tensor.transpose(tps[:, 0:128], vt_sb[:, i * 128 : (i + 1) * 128],
                                    ident)
                nc.vector.tensor_copy(v_sb[:, tt, 0:64], tps[:, 0:64])
                nc.vector.tensor_copy(v_sb[:, tt, 65:129], tps[:, 64:128])

        a1ctx.close()
        pq = actx.enter_context(tc.tile_pool(name="pq", bufs=1))
        wq_sb = pq.tile([128, KT, 512], f32r)
        nc.sync.dma_start(out=wq_sb, in_=wqT.rearrange("(t p) e -> p t e", p=128))

        for sc in range(NSCA):
            sl = slice(sc * SCA, (sc + 1) * SCA)
            hs_sb = hsp.tile([128, KT, SCA], f32r, name="hs_sb")
            nc.sync.dma_start(
                out=hs_sb, in_=hsT[:, sl].rearrange("(t p) s -> p t s", p=128))
            for jj in range(2):  # two psum tiles, 2 e-tiles each
                q_ps = sp.tile([128, 2, 512], f32, name="q_ps", tag="sp")
                for half in range(2):
                    j = jj * 2 + half
                    dst_sl = slice(half * SCA, (half + 1) * SCA)
                    for ki in range(KT):
                        nc.tensor.matmul(
                            q_ps[:, 0, dst_sl],
                            wq_sb[:, ki, j * 128 : (j + 1) * 128],
                            hs_sb[:, ki, :], start=ki == 0, stop=ki == KT - 1)
                    rope_drain(q_ps[:, 0, dst_sl], q_ps[:, 1, dst_sl],
                               qT_sb[:, j, :], sl)

        actx.close()  # release phase-A pools so phase-C weights can reuse SBUF

        # ---------------- phase B: attention per (q-tile pair j, s-chunk) ------
        for j in range(4):
            for sc in range(NSCB):
                sl = slice(sc * SCB, (sc + 1) * SCB)
                qA = qT_sb[0:64, j, sl]
                qB = qT_sb[64:128, j, sl]
                oA = op.tile([128, 512], f32, name="oA", tag="o")
                oB = op.tile([128, 512], f32, name="oB", tag="o")
                for g in range(TT // 2):
                    sA = sp.tile([128, 2, 512], f32, name="sA", tag="sp")
                    sB = sp.tile([128, 2, 512], f32, name="sB", tag="sp")
                    for i in range(2):
                        tt = 2 * g + i
                        ksl = slice(tt * 128, (tt + 1) * 128)
                        nc.tensor.matmul(sA[:, i, :], kT_sb[0:64, ksl], qA,
                                         start=True, stop=True,
                                         tile_position=(0, 0))
                        nc.tensor.matmul(sB[:, i, :], kT_sb[64:128, ksl], qB,
                                         start=True, stop=True,
                                         tile_position=(64, 0))
                    pA = ptp.tile([128, 2, 512], f32r, name="pA", tag="pt")
                    pB = ptp.tile([128, 2, 512], f32r, name="pB", tag="pt")
                    nc.scalar.activation(pA, sA, Exp, scale=0.125)
                    nc.scalar.activation(pB, sB, Exp, scale=0.125)
                    for i in range(2):
                        tt = 2 * g + i
                        st = tt == 0
                        sp_ = tt == TT - 1
                        nc.tensor.matmul(oA[0:65, :], v_sb[:, tt, 0:65],
                                         pA[:, i, :], start=st, stop=sp_)
                        nc.tensor.matmul(oB[0:65, :], v_sb[:, tt, 65:130],
                                         pB[:, i, :], start=st, stop=sp_)
                # normalize: oT = o_unnorm / rowsum (rowsum at psum partition 64)
                rr = rrp.tile([128, 2, 512], f32, name="rr", bufs=1)
                nc.vector.tensor_copy(rr[64:65, 0, :], oA[64:65, :])
                nc.vector.tensor_copy(rr[64:65, 1, :], oB[64:65, :])
                rr0 = rrp.tile([128, 2, 512], f32, name="rr0", bufs=1)
                nc.sync.dma_start(out=rr0[0:1, :, :], in_=rr[64:65, :, :])
                rrec = rrp.tile([128, 2, 512], f32r, name="rrec", bufs=1)
                with nc.allow_low_precision(reason="tf32 rowsum recip is plenty"):
                    nc.vector.reciprocal(rrec[0:1, 0, :], rr0[0:1, 0, :])
                    nc.vector.reciprocal(rrec[0:1, 1, :], rr0[0:1, 1, :])
                bc = sp.tile([128, 2, 512], f32, name="bc", tag="sp")
                nc.tensor.matmul(bc[:, 0, :], ones_sb[0:1, 0:128],
                                 rrec[0:1, 0, :], start=True, stop=False)
                nc.tensor.matmul(bc[:, 0, :], ones_sb[0:1, 128:256],
                                 rrec[0:1, 1, :], start=False, stop=True)
                nc.vector.tensor_copy(oT_sb[0:64, j, sl], oA[0:64, :])
                ob_sb = obst.tile([64, 512], f32r, name="ob_sb")
                nc.vector.tensor_copy(ob_sb, oB[0:64, :])
                nc.sync.dma_start(out=oT_sb[64:128, j, sl], in_=ob_sb)
                nc.vector.tensor_tensor(out=oT_sb[:, j, sl], in0=oT_sb[:, j, sl],
                                        in1=bc[:, 0, :], op=mult)

        # ---------------- phase C: o_proj ----------------
        pc = ctx.enter_context(tc.tile_pool(name="pc", bufs=1))
        wo_sb = pc.tile([128, 4, HID], f32r)
        nc.sync.dma_start(out=wo_sb, in_=woT.rearrange("(t p) h -> p t h", p=128))
        ostg = ctx.enter_context(tc.tile_pool(name="ostg", bufs=3))
        for st in range(S // 128):
            ssl = slice(st * 128, (st + 1) * 128)
            for hc in range(HID // 512):
                hsl = slice(hc * 512, (hc + 1) * 512)
                ops = op.tile([128, 512], f32, name="ops", tag="o")
                for et in range(4):
                    nc.tensor.matmul(ops, oT_sb[:, et, ssl],
                                     wo_sb[:, et, hsl],
                                     start=et == 0, stop=et == 3)
                og = ostg.tile([128, 512], f32, name="og")
                nc.vector.tensor_copy(og, ops)
                nc.sync.dma_start(out=out[ssl, hsl], in_=og)

    nc.finalize()
    return nc


def _get_nc():
    if "nc" not in _NC_CACHE:
        _NC_CACHE["nc"] = _build_nc()
    return _NC_CACHE["nc"]


def _rot_matrix():
    # R @ q = rotate_half(q) per 64-block: R[i, i+32] = -1 (i%64<32),
    # R[i, i-32] = +1 (i%64>=32). Device needs lhsT = R.T.
    R = np.zeros((128, 128), dtype=np.float32)
    for blk in (0, 64):
        for i in range(32):
            R[blk + i, blk + i + 32] = -1.0
            R[blk + 32 + i, blk + i] = 1.0
    return np.ascontiguousarray(R.T)


def _marshal(inputs):
    hs = np.asarray(inputs["hidden_states"], dtype=np.float32)
    cos = np.asarray(inputs["cos"], dtype=np.float32)
    sin = np.asarray(inputs["sin"], dtype=np.float32)
    Wq = np.asarray(inputs["Wq"], dtype=np.float32)
    Wk = np.asarray(inputs["Wk"], dtype=np.float32)
    Wv = np.asarray(inputs["Wv"], dtype=np.float32)
    Wo = np.asarray(inputs["Wo"], dtype=np.float32)

    c = np.ascontiguousarray
    rotm = _rot_matrix()
    in_maps = []
    for core in range(NCORES):
        b, kg = divmod(core, 4)
        gheads = [kg * 8 + l for l in PERM_LOCAL]
        kvh = [2 * kg, 2 * kg + 1]
        wqT = c(Wq.reshape(H, D, HID)[gheads].reshape(512, HID).T)
        wkT = c(Wk.reshape(KVH, D, HID)[kvh].reshape(128, HID).T)
        wvT = c(Wv.reshape(KVH, D, HID)[kvh].reshape(128, HID).T)
        woT = c(Wo.T.reshape(H, D, HID)[gheads].reshape(512, HID))
        hsT = c(hs[b].T)
        cosT = cos[b].T  # [64, S]
        sinT = sin[b].T
        cosT2 = c(np.concatenate([cosT, cosT], axis=0))
        sinT2 = c(np.concatenate([sinT, sinT], axis=0))
        in_maps.append({
            "hsT": hsT, "cosT2": cosT2, "sinT2": sinT2, "rotm": rotm,
            "wqT": wqT, "wkT": wkT, "wvT": wvT, "woT": woT,
        })
    return in_maps


def run(inputs, trace=False, trace_cores=None):
    from concourse.bass_utils import run_bass_kernel_spmd

    nc = _get_nc()
    in_maps = _marshal(inputs)
    res = run_bass_kernel_spmd(
        nc, in_maps, core_ids=list(range(NCORES)), trace=trace,
        trace_cores=trace_cores)
    outs = [res.results[i]["out"] for i in range(NCORES)]
    final = np.zeros((B, S, HID), dtype=np.float32)
    for b in range(B):
        final[b] = outs[4 * b] + outs[4 * b + 1] + outs[4 * b + 2] + outs[4 * b + 3]
    return final, res


def kernel(**inputs):
    out, _ = run(inputs, trace=False)
    return out



# revision 12
# speedup vs baseline: 5.4953x; 5.4953x over previous
"""Trainium2 Bass kernel for GQA multi-head attention (B=2,S=2048,HID=2048,H=32,KVH=8,D=64).

Sharding: 8 cores = 2 (batch) x 4 (s-quarters). Each core handles one batch
element and a 512-row slice of s: it computes K/V projections + RoPE for the
FULL sequence (so attention has every key/value), Q projection + attention +
o_proj only for its own 512 query rows, and writes a disjoint [512, HID]
bf16 slice of the final output. Host just concatenates (no partial sums) --
this keeps per-iteration output traffic at the 2.1 MB/core minimum, which
dominates the measured dispatch time on this runtime.

All matmul operands are bf16 (PSUM accumulation stays fp32); tolerance is
2e-2 relative L2 and bf16 end-to-end lands ~1e-3.

Device layouts (host pre-marshalled, all bf16):
  hsT   [HID, S]    transposed hidden states for this core's batch
  hsTq  [HID, 512]  own s-slice columns of hsT
  cosT2 [128, S]    cos.T stacked twice (RoPE per 64-row head half)
  sinT2 [128, S]    sin.T stacked twice
  cosTq/sinTq       own s-slice columns of cosT2/sinT2
  rotm  [128, 128]  R.T with R q = rotate_half(q) per 64-block
  wqT   [HID, HID]  Wq rows permuted to head-tile order, transposed
  wkT   [HID, 512]  Wk.T (kv head pairs are naturally 128-row tiles)
  wvT   [HID, 512]  Wv.T
  woT   [HID, HID]  Wo columns in the same permuted feature order
  out   [512, HID]  this core's slice of the output (bf16)

Head tile order: tile j (j=0..15) holds q heads (8p+jj, 8p+4+jj) where
p=j//4, jj=j%4 -- partitions 0:64 use kv head 2p, 64:128 use kv head 2p+1,
matching the kv-pair tile layout so score matmuls can pack both heads into
the PE array halves via tile_position.

Softmax: scores are O(10) so exp without max-subtraction is safe in fp32;
row sums ride as ones-columns in the attn@V lhsT (partition 64 for head A,
partition 63 for head B -- partition-aligned so no SBUF shifts), and
normalization is reciprocal + one PE outer-product broadcast + DVE multiply.
"""

import sys

if "/opt/trn_rl_repo" not in sys.path:
    sys.path.insert(0, "/opt/trn_rl_repo")

import numpy as np

B, S, HID = 2, 2048, 2048
H, KVH, D = 32, 8, 64
NCORES = 8
SQ = S // 4  # 512 query rows per core

# tile j holds q heads (8p+jj, 8p+4+jj), p=j//4, jj=j%4
PERM_HEADS = []
for _p in range(4):
    for _jj in range(4):
        PERM_HEADS += [8 * _p + _jj, 8 * _p + 4 + _jj]

_NC_CACHE = {}


def _build_nc():
    import concourse.bass as bass
    import concourse.mybir as mybir
    from concourse import bacc
    from concourse.tile import TileContext
    from concourse.masks import make_identity
    from contextlib import ExitStack

    f32 = mybir.dt.float32
    f32r = mybir.dt.float32r
    bf16 = mybir.dt.bfloat16
    Exp = mybir.ActivationFunctionType.Exp
    mult = mybir.AluOpType.mult
    add = mybir.AluOpType.add

    nc = bacc.Bacc(None, target_bir_lowering=False)

    hsT = nc.declare_dram_parameter("hsT", [HID, S], bf16, isOutput=False)
    hsTq = nc.declare_dram_parameter("hsTq", [HID, SQ], bf16, isOutput=False)
    cosT2 = nc.declare_dram_parameter("cosT2", [128, S], bf16, isOutput=False)
    sinT2 = nc.declare_dram_parameter("sinT2", [128, S], bf16, isOutput=False)
    cosTq = nc.declare_dram_parameter("cosTq", [128, SQ], bf16, isOutput=False)
    sinTq = nc.declare_dram_parameter("sinTq", [128, SQ], bf16, isOutput=False)
    rotm = nc.declare_dram_parameter("rotm", [128, 128], bf16, isOutput=False)
    wqT = nc.declare_dram_parameter("wqT", [HID, HID], bf16, isOutput=False)
    wkT = nc.declare_dram_parameter("wkT", [HID, 512], bf16, isOutput=False)
    wvT = nc.declare_dram_parameter("wvT", [HID, 512], bf16, isOutput=False)
    woT = nc.declare_dram_parameter("woT", [HID, HID], bf16, isOutput=False)
    out = nc.declare_dram_parameter("out", [SQ, HID], bf16, isOutput=True)

    KT = HID // 128  # 16 contraction k-tiles for projections
    TT = S // 128    # 16 key tiles
    CH = 256         # K/V pass s-chunk width
    NCH = S // CH    # 8

    with TileContext(nc) as tc, ExitStack() as ctx:
        # ---- persistent tiles (live across phases) ----
        persist = ctx.enter_context(tc.tile_pool(name="persist", bufs=1))
        kT_sb = persist.tile([128, 4, S], bf16)        # k' per kv pair
        v_sb = persist.tile([128, 4, TT, 130], bf16)   # v[t,d] + ones cols
        qT_sb = persist.tile([128, 16, SQ], bf16)      # q' per head tile
        oT_sb = persist.tile([128, 16, SQ], bf16)      # normalized attn out^T
        rot_sb = persist.tile([128, 128], bf16)
        nmask = persist.tile([128, 256], f32r)         # bc broadcast masks
        cos_sb = persist.tile([128, S], bf16)
        sin_sb = persist.tile([128, S], bf16)

        nc.sync.dma_start(out=rot_sb, in_=rotm[:, :])
        nc.sync.dma_start(out=cos_sb, in_=cosT2[:, :])
        nc.sync.dma_start(out=sin_sb, in_=sinT2[:, :])
        onesf = persist.tile([128, 64], bf16)
        nc.vector.memset(onesf, 1.0)
        # nmask row 64: cols 0:64 select head A partitions, cols 128:192 head B
        nmaskf = persist.tile([128, 256], f32)
        nc.vector.memset(nmaskf, 0.0)
        nc.vector.memset(nmaskf[64:65, 0:64], 1.0)
        nc.vector.memset(nmaskf[64:65, 128:192], 1.0)
        nc.vector.tensor_copy(nmask, nmaskf)
        nc.vector.tensor_copy(v_sb[:, :, :, 64], onesf[:, 0:64])
        nc.vector.tensor_copy(v_sb[:, :, :, 129], onesf[:, 0:64])

        # ---------------- phase A1: K/V projections + RoPE(k) + v layout ----
        actx = ExitStack()
        pkv = actx.enter_context(tc.tile_pool(name="pkv", bufs=1))
        wk_sb = pkv.tile([128, KT, 512], bf16)
        wv_sb = pkv.tile([128, KT, 512], bf16)
        ident = pkv.tile([128, 128], bf16)
        nc.sync.dma_start(out=wk_sb, in_=wkT.rearrange("(t p) e -> p t e", p=128))
        nc.sync.dma_start(out=wv_sb, in_=wvT.rearrange("(t p) e -> p t e", p=128))
        make_identity(nc, ident)

        hsp = actx.enter_context(tc.tile_pool(name="hsp", bufs=3))
        ropep = actx.enter_context(tc.tile_pool(name="ropep", bufs=2))
        vstg = actx.enter_context(tc.tile_pool(name="vstg", bufs=2))

        # PSUM A1: kv tile 4 banks x1, rope scratch 1x2, transpose 1x2 = 8
        kvp = actx.enter_context(tc.tile_pool(name="kvp", bufs=1, space="PSUM"))
        rp = actx.enter_context(tc.tile_pool(name="rp", bufs=2, space="PSUM"))
        tp = actx.enter_context(tc.tile_pool(name="tp", bufs=2, space="PSUM"))

        def rope_drain(ps, rot_ps, stage_t, csl, ssl, dst):
            """dst = ps*cos[:, csl] + (R@ps)*sin[:, csl] (widths match)."""
            nc.scalar.copy(stage_t, ps)
            nc.tensor.matmul(rot_ps, rot_sb, stage_t, start=True, stop=True)
            nc.vector.tensor_tensor(out=dst, in0=stage_t, in1=csl, op=mult)
            shs = ropep.tile(stage_t.shape, bf16, name="shs", tag="shs")
            nc.vector.tensor_tensor(out=shs, in0=rot_ps, in1=ssl, op=mult)
            nc.vector.tensor_tensor(out=dst, in0=dst, in1=shs, op=add)

        for sc in range(NCH):
            sl = slice(sc * CH, (sc + 1) * CH)
            hs_sb = hsp.tile([128, KT, CH], bf16, name="hs_sb")
            nc.sync.dma_start(
                out=hs_sb, in_=hsT[:, sl].rearrange("(t p) s -> p t s", p=128))
            kv_ps = kvp.tile([128, 8, CH], f32, name="kv_ps")
            for ft in range(4):
                for ki in range(KT):
                    nc.tensor.matmul(kv_ps[:, ft, :],
                                     wk_sb[:, ki, ft * 128:(ft + 1) * 128],
                                     hs_sb[:, ki, :],
                                     start=ki == 0, stop=ki == KT - 1)
            for ft in range(4):
                for ki in range(KT):
                    nc.tensor.matmul(kv_ps[:, 4 + ft, :],
                                     wv_sb[:, ki, ft * 128:(ft + 1) * 128],
                                     hs_sb[:, ki, :],
                                     start=ki == 0, stop=ki == KT - 1)
            # k: RoPE into kT_sb per kv pair ft
            for ft in range(4):
                rot_ps = rp.tile([128, CH], f32, name="rot_ps")
                kst = ropep.tile([128, CH], bf16, name="kst", tag="kst")
                rope_drain(kv_ps[:, ft, :], rot_ps, kst,
                           cos_sb[:, sl], sin_sb[:, sl], kT_sb[:, ft, sl])
            # v: stage to SBUF, transpose 128-blocks into [t, d] layout
            vt_sb = vstg.tile([128, 4, CH], bf16, name="vt_sb")
            nc.scalar.copy(vt_sb, kv_ps[:, 4:8, :])
            for ft in range(4):
                for i in range(CH // 128):
                    tt = (sc * CH) // 128 + i
                    tps = tp.tile([128, 128], bf16, name="tps")
                    nc.tensor.transpose(tps, vt_sb[:, ft, i * 128:(i + 1) * 128],
                                        ident)
                    nc.vector.tensor_copy(v_sb[:, ft, tt, 0:64], tps[:, 0:64])
                    nc.vector.tensor_copy(v_sb[:, ft, tt, 65:129], tps[:, 64:128])
        actx.close()

        # ---------------- phase A2: Q projection + RoPE for own s-slice -----
        a2ctx = ExitStack()
        pq = a2ctx.enter_context(tc.tile_pool(name="pq", bufs=1))
        wq_sb = pq.tile([128, KT, HID], bf16)
        nc.sync.dma_start(out=wq_sb, in_=wqT.rearrange("(t p) e -> p t e", p=128))
        hsq = a2ctx.enter_context(tc.tile_pool(name="hsq", bufs=1))
        hs_q = hsq.tile([128, KT, SQ], bf16)
        nc.sync.dma_start(
            out=hs_q, in_=hsTq.rearrange("(t p) s -> p t s", p=128))
        cosq_sb = hsq.tile([128, SQ], bf16)
        sinq_sb = hsq.tile([128, SQ], bf16)
        nc.sync.dma_start(out=cosq_sb, in_=cosTq[:, :])
        nc.sync.dma_start(out=sinq_sb, in_=sinTq[:, :])
        ropeq = a2ctx.enter_context(tc.tile_pool(name="ropeq", bufs=2))
        qp = a2ctx.enter_context(tc.tile_pool(name="qp", bufs=3, space="PSUM"))
        rq = a2ctx.enter_context(tc.tile_pool(name="rq", bufs=2, space="PSUM"))

        for et in range(16):
            q_ps = qp.tile([128, SQ], f32, name="q_ps")
            for ki in range(KT):
                nc.tensor.matmul(q_ps, wq_sb[:, ki, et * 128:(et + 1) * 128],
                                 hs_q[:, ki, :], start=ki == 0, stop=ki == KT - 1)
            rot_ps = rq.tile([128, SQ], f32, name="rot_ps")
            qst = ropeq.tile([128, SQ], bf16, name="qst", tag="qst")
            nc.scalar.copy(qst, q_ps)
            nc.tensor.matmul(rot_ps, rot_sb, qst, start=True, stop=True)
            nc.vector.tensor_tensor(out=qT_sb[:, et, :], in0=qst,
                                    in1=cosq_sb, op=mult)
            shs = ropeq.tile([128, SQ], bf16, name="qshs", tag="qshs")
            nc.vector.tensor_tensor(out=shs, in0=rot_ps, in1=sinq_sb, op=mult)
            nc.vector.tensor_tensor(out=qT_sb[:, et, :], in0=qT_sb[:, et, :],
                                    in1=shs, op=add)
        a2ctx.close()

        # ---------------- phase B: attention per head tile j ----------------
        bctx = ExitStack()
        # prefetch o_proj weights during attention (DMA overlaps Act-bound B);
        # pool lives in the outer ctx because phase C still reads it
        pwo = ctx.enter_context(tc.tile_pool(name="pwo", bufs=1))
        wo_sb = pwo.tile([128, KT, HID], bf16)
        nc.sync.dma_start(out=wo_sb, in_=woT.rearrange("(t p) h -> p t h", p=128))

        # PSUM B: scores 2 banks x2, oAB 2 banks x1, bc 2 banks x1 = 8
        sp = bctx.enter_context(tc.tile_pool(name="sp", bufs=2, space="PSUM"))
        op = bctx.enter_context(tc.tile_pool(name="op", bufs=1, space="PSUM"))
        bp = bctx.enter_context(tc.tile_pool(name="bp", bufs=1, space="PSUM"))
        ptp = bctx.enter_context(tc.tile_pool(name="ptp", bufs=3))
        nrm = bctx.enter_context(tc.tile_pool(name="nrm", bufs=2))

        for j in range(16):
            p = j // 4
            qA = qT_sb[0:64, j, :]
            qB = qT_sb[64:128, j, :]
            oAB = op.tile([128, 2, SQ], f32, name="oAB")

            def scores(tt):
                sAB = sp.tile([128, 2, SQ], f32, name="sAB", tag="sp")
                ksl = slice(tt * 128, (tt + 1) * 128)
                nc.tensor.matmul(sAB[:, 0, :], kT_sb[0:64, p, ksl], qA,
                                 start=True, stop=True, tile_position=(0, 0))
                nc.tensor.matmul(sAB[:, 1, :], kT_sb[64:128, p, ksl], qB,
                                 start=True, stop=True, tile_position=(64, 0))
                pAB = ptp.tile([128, 2, SQ], bf16, name="pAB", tag="pt")
                nc.scalar.activation(pAB, sAB, Exp, scale=0.125)
                return pAB

            def attnv(tt, pAB):
                st = tt == 0
                sp_ = tt == TT - 1
                nc.tensor.matmul(oAB[0:65, 0, :], v_sb[:, p, tt, 0:65],
                                 pAB[:, 0, :], start=st, stop=sp_)
                nc.tensor.matmul(oAB[0:65, 1, :], v_sb[:, p, tt, 65:130],
                                 pAB[:, 1, :], start=st, stop=sp_)

            # software pipeline: scores one tt ahead of attnv
            pprev = scores(0)
            for tt in range(1, TT):
                pnext = scores(tt)
                attnv(tt - 1, pprev)
                pprev = pnext
            attnv(TT - 1, pprev)

            # normalize: both rowsums sit at partition 64 (ones columns)
            rrec = nrm.tile([128, 2, SQ], f32r, name="rrec")
            with nc.allow_low_precision(reason="tf32 rowsum recip is plenty"):
                nc.vector.reciprocal(rrec[64:65, 0, :], oAB[64:65, 0, :])
                nc.vector.reciprocal(rrec[64:65, 1, :], oAB[64:65, 1, :])
            bc = bp.tile([128, 2, SQ], f32, name="bc")
            nc.tensor.matmul(bc[:, 0, :], nmask[64:65, 0:128],
                             rrec[64:65, 0, :], start=True, stop=True)
            nc.tensor.matmul(bc[:, 1, :], nmask[64:65, 128:256],
                             rrec[64:65, 1, :], start=True, stop=True)
            bc_sb = nrm.tile([128, 2, SQ], f32, name="bc_sb")
            nc.vector.tensor_copy(bc_sb, bc)
            nc.vector.tensor_tensor(out=oT_sb[0:64, j, :], in0=oAB[0:64, 0, :],
                                    in1=bc_sb[0:64, 0, :], op=mult)
            ob_sb = nrm.tile([64, SQ], bf16, name="ob_sb")
            nc.vector.tensor_tensor(out=ob_sb, in0=oAB[0:64, 1, :],
                                    in1=bc_sb[0:64, 1, :], op=mult)
            nc.sync.dma_start(out=oT_sb[64:128, j, :], in_=ob_sb)
        bctx.close()

        # ---------------- phase C: o_proj for own s-slice -------------------
        cctx = ExitStack()
        dp = cctx.enter_context(tc.tile_pool(name="dp", bufs=4, space="PSUM"))
        ostg = cctx.enter_context(tc.tile_pool(name="ostg", bufs=3))
        for st in range(SQ // 128):
            ssl = slice(st * 128, (st + 1) * 128)
            for hc in range(HID // 512):
                hsl = slice(hc * 512, (hc + 1) * 512)
                ops = dp.tile([128, 512], f32, name="ops")
                for et in range(16):
                    nc.tensor.matmul(ops, oT_sb[:, et, ssl],
                                     wo_sb[:, et, hsl],
                                     start=et == 0, stop=et == 15)
                og = ostg.tile([128, 512], bf16, name="og")
                nc.vector.tensor_copy(og, ops)
                nc.sync.dma_start(out=out[ssl, hsl], in_=og)
        cctx.close()

    nc.finalize()
    return nc


def _get_nc():
    if "nc" not in _NC_CACHE:
        _NC_CACHE["nc"] = _build_nc()
    return _NC_CACHE["nc"]


def _rot_matrix():
    # R @ q = rotate_half(q) per 64-block: R[i, i+32] = -1 (i%64<32),
    # R[i, i-32] = +1 (i%64>=32). Device needs lhsT = R.T.
    R = np.zeros((128, 128), dtype=np.float32)
    for blk in (0, 64):
        for i in range(32):
            R[blk + i, blk + i + 32] = -1.0
            R[blk + 32 + i, blk + i] = 1.0
    return np.ascontiguousarray(R.T)


def _marshal(inputs):
    import ml_dtypes

    bf16 = ml_dtypes.bfloat16

    hs = np.asarray(inputs["hidden_states"], dtype=np.float32)
    cos = np.asarray(inputs["cos"], dtype=np.float32)
    sin = np.asarray(inputs["sin"], dtype=np.float32)
    Wq = np.asarray(inputs["Wq"], dtype=np.float32)
    Wk = np.asarray(inputs["Wk"], dtype=np.float32)
    Wv = np.asarray(inputs["Wv"], dtype=np.float32)
    Wo = np.asarray(inputs["Wo"], dtype=np.float32)

    def c(a):
        return np.ascontiguousarray(a).astype(bf16)

    perm = PERM_HEADS
    rotm = c(_rot_matrix())
    # Wq rows (out features) reordered to head-tile order, then transposed
    wqT = c(Wq.reshape(H, D, HID)[perm].reshape(HID, HID).T)
    wkT = c(Wk.T)
    wvT = c(Wv.T)
    # Wo columns (in features) in the same permuted order
    woT = c(Wo.T.reshape(H, D, HID)[perm].reshape(HID, HID))

    in_maps = []
    for core in range(NCORES):
        b, q = divmod(core, 4)
        ssl = slice(q * SQ, (q + 1) * SQ)
        hsTb = c(hs[b].T)
        cosT = cos[b].T  # [64, S]
        sinT = sin[b].T
        cosT2 = c(np.concatenate([cosT, cosT], axis=0))
        sinT2 = c(np.concatenate([sinT, sinT], axis=0))
        in_maps.append({
            "hsT": hsTb,
            "hsTq": np.ascontiguousarray(hsTb[:, ssl]),
            "cosT2": cosT2, "sinT2": sinT2,
            "cosTq": np.ascontiguousarray(cosT2[:, ssl]),
            "sinTq": np.ascontiguousarray(sinT2[:, ssl]),
            "rotm": rotm,
            "wqT": wqT, "wkT": wkT, "wvT": wvT, "woT": woT,
        })
    return in_maps


def run(inputs, trace=False, trace_cores=None):
    from concourse.bass_utils import run_bass_kernel_spmd

    nc = _get_nc()
    in_maps = _marshal(inputs)
    res = run_bass_kernel_spmd(
        nc, in_maps, core_ids=list(range(NCORES)), trace=trace,
        trace_cores=trace_cores)
    final = np.zeros((B, S, HID), dtype=np.float32)
    for core in range(NCORES):
        b, q = divmod(core, 4)
        final[b, q * SQ:(q + 1) * SQ, :] = np.asarray(
            res.results[core]["out"], dtype=np.float32)
    return final, res


def kernel(**inputs):
    out, _ = run(inputs, trace=False)
    return out
